# revision 52
# baseline (speedup 1.0000x reference)
"""AVAttention Trainium2 kernel (8 NeuronCores, Bass/Tile).

Reference computation per sample b:
    k   = ph @ Wk + bk                  [S, D]
    q   = g.reshape(CF, T).T @ Wq + bq  [T, D]
    att = softmax(mask(q @ k.T / sqrt(D)))  over S (mask from lengths[b])
    out = (att @ (ph @ Wv + bv)) @ Wmel + bmel -> [64, F, T]

Sharding: data-parallel over batch B=16 across 8 cores, 2 samples per
core (paired long+short by lengths), weights replicated, outputs
concatenated.  No collectives.

Layout: everything on-chip is "transposed" (feature dim on partitions)
so no attention-matrix transposes are ever needed:
    kq[p,t], exps[s,t], valueT[d,t], outT[m,t]

Algebraic folds (host-side, exact):
  * logits = phT^T @ (Wqk^T @ g) with Wqk = Wq @ Wk^T prefolded; bq
    enters as kq += Wk@bq; bk cancels in softmax.
  * v = ph @ Wv on host; bv folded into bmel.
  * short samples: vmel = v @ Wmel prefolded on host, so the value and
    output GEMMs collapse into one: out = (att/den) @ vmel + bmel.
  * samples with len < 128 are computed exactly on the host (f32); the
    device slot still runs but its output is discarded.  This removes
    the error-critical tiny-length samples from the fp8 budget.

Precision: EVERY device GEMM runs fp8e4m3 DoubleRow (2 contraction
rows per PE pass = 2x bf16 throughput): kq (wqk8 x g8), attention
(phT8 x kq8), denominator (vcol8 x exps8), value (v8 x exps8, long
slot), out (wmel8 x valT8 long / vmel8 x exn8 short).  f32 PSUM
everywhere.  Post-softmax fp8 error scales ~1/sqrt(len), so with
len<128 hosted the emulated end-to-end absmax-relative error is
~1.4e-2 vs the 2e-2 gate (host emulator emulate.py, validated to ~4%
against HW on the previous all-bf16+partial-fp8 kernel).

Masking costs nothing: exp() drains with a per-partition bias column
(0 valid / -100 invalid) so invalid positions give exp(-100)=0 exactly;
den then needs no validity column and value/vmel need no zeroed rows.

Scales (all exact powers of two, folded into constants):
  wqk8 = fp8(64*Wqk)           exp scale = SCALE/64
  v8 = fp8(64*v), valT8 = fp8(64*value): vcol8 long entries = 1
  wmel8 = fp8(64*Wmel): long out drain scale = 1/4096
  vmel8 = fp8(16*vmel), exn8 = fp8(64*att/den): vcol8 short = 1/64,
  short out drain scale = 1/1024

Softmax denominator: the den matmul's weights (vcol8) are REPLICATED
across 128 columns so den lands on all PSUM partitions; 1/den is then
a single full-width reciprocal_approx_fast (the plain DVE reciprocal
costs 3.3us regardless of shape; approx is ~5x faster at 18 bits).

Schedule (engines run their queues in order, so emission order = the
dependency schedule): per-unit emission software-pipelines across
units -- prev's value stage (val matmuls sandwiching den+recip so the
den matmuls' exp-drain waits are covered and recip precedes the val
drains in the DVE queue) runs before the current unit's kq; prev's
out pairs interleave with current att pairs; short-unit den lags two
steps behind att under leftover out pairs; the final unit runs
recip/exn inside the interleave so prev's last out pairs cover the
chain.  The val matmuls touch the last exps pair as late as possible
(its exp drain races them at the unit boundary).  kq/out drains
alternate Vector/Scalar.  All matmul PSUMs are [128, 2, TC] tiles (2
banks) rotating 3-deep.  g loads on the Scalar engine's DMA queue.
Output stores in (f h t) bf16, host swaps axes and upcasts.  8
memset-fed warm-up matmuls release the HAM clock throttle during the
initial DMA wait.

Measured on 8 trn2 cores (seed-0 inputs, A=8/BB=4): 186.0us, rel err
1.401e-2 (= the emulate.py prediction; gate 2e-2).  History: 542us ->
354 -> 315 (prior sessions, partial fp8) -> 209 (all-fp8 DR + host
len<128 + fold) -> 189 (replicated den + fast recip + val-first
order) -> 186 (deferred den, val pair ordering, warm-up trim).
PE ~81% busy at the 216ns/matmul stream floor, ~672 matmul slots.
"""

import math

import numpy as np

import concourse.bacc as bacc
import concourse.mybir as mybir
import concourse.tile as tile
from concourse.bass_utils import run_bass_kernel_spmd

B, S, T = 16, 1024, 2048
CF = 2560          # q in_features = C*Fdim = 128*20
KO = CF // 128     # 20 contraction chunks for the kq projection
D = 512            # out_dim (k/q/v width); 4 partition blocks
PH = 512           # ph feature dim; 4 partition blocks
MEL = 1280         # out features; 10 partition blocks
N_CORES = 8
B_LOC = B // N_CORES
SCALE = 1.0 / math.sqrt(D)

TC = 512
NTC = T // TC
NPO = PH // 128
NDB = D // 128
NMB = MEL // 128

F32 = mybir.dt.float32
BF = mybir.dt.bfloat16
F8 = mybir.dt.float8e4
DR = mybir.MatmulPerfMode.DoubleRow

# power-of-two scales keeping every fp8 tensor in e4m3's normal range
W8S = 64.0          # wqk, kq
VS = 64.0           # v8 = fp8(VS*v); == VOS so valT8 = pv2 * (1/den) exactly
VOS = 64.0          # valT8 = fp8(VOS*value)
WMS = 64.0          # wmel8 = fp8(WMS*Wmel); long out scale 1/(VOS*WMS)
VMS = 16.0          # vmel8 = fp8(VMS*vmel)
EXS = 64.0          # exn8 = fp8(EXS*att/den); vcol8 short = 1/EXS
HOST_MAX_LEN = 127  # samples this short are computed on the host

_NC_CACHE = {}


def _build_nc(A, BB, FV=(0, 0)):
    NSBT = A + BB            # total packed s blocks
    SPK = NSBT * 128
    trips = (A, BB)          # att/val block count per sample slot
    SOFF = (0, A)            # block offset of each sample's region
    # FV[b]: number of leading s-blocks fully valid on EVERY core for
    # slot b -> their exp drains can go out as one 2-block activation
    # with a shared (all-zero) bias column.

    nc = bacc.Bacc("TRN2", target_bir_lowering=False,
                   dynamic_dma_scratch_size=256)

    phT8_d = nc.dram_tensor("phT8", [128, NPO, SPK], F8, kind="ExternalInput")
    g8_d = nc.dram_tensor("g8", [B_LOC, NTC, 128, KO, TC], F8, kind="ExternalInput")
    wqk8_d = nc.dram_tensor("wqk8", [128, KO, PH], F8, kind="ExternalInput")
    v8_d = nc.dram_tensor("v8", [128, A, D], F8, kind="ExternalInput")
    vmel8_d = nc.dram_tensor("vmel8", [128, BB, MEL], F8, kind="ExternalInput")
    wmel8_d = nc.dram_tensor("wmel8", [128, NDB, MEL], F8, kind="ExternalInput")
    bmel_d = nc.dram_tensor("bmel", [128, NMB], F32, kind="ExternalInput")
    kqb_d = nc.dram_tensor("kqb", [128, NPO], F32, kind="ExternalInput")
    # [128, NSBT, 128]: den scale replicated across 128 columns, so the
    # den matmul writes its result to ALL output partitions: reciprocal
    # then runs on [128, TC] (full DVE width) and no partition broadcast
    # is needed.  The 128B pair stride also satisfies the s3 ldweights
    # DoubleRow alignment restriction.
    vcol8_d = nc.dram_tensor("vcol8", [128, NSBT, 128], F8, kind="ExternalInput")
    ebias_d = nc.dram_tensor("ebias", [128, NSBT], F32, kind="ExternalInput")
    out_d = nc.dram_tensor("out", [B_LOC, 20, 64, T], BF, kind="ExternalOutput")

    with tile.TileContext(nc) as tc:
        with tc.tile_pool(name="const", bufs=1) as cpool, \
             tc.tile_pool(name="sb", bufs=2) as pool, \
             tc.tile_pool(name="ps", bufs=2, space="PSUM") as ps:

            # ---- constants / weights (resident) ----
            # wqk8 in halves so the first kq group starts after half 0
            # lands; everything else arrives under unit 0.
            KH = KO // 2
            wqk8_t = cpool.tile([128, KO, PH], F8)
            nc.sync.dma_start(wqk8_t[:, 0:KH, :], wqk8_d[:, 0:KH, :])
            kqb_t = cpool.tile([128, NPO], F32)
            nc.sync.dma_start(kqb_t[:], kqb_d[:])
            nc.sync.dma_start(wqk8_t[:, KH:KO, :], wqk8_d[:, KH:KO, :])
            vcol8_t = cpool.tile([128, NSBT, 128], F8)
            nc.sync.dma_start(vcol8_t[:], vcol8_d[:])
            ebias_t = cpool.tile([128, NSBT], F32)
            nc.sync.dma_start(ebias_t[:], ebias_d[:])
            wmel8_t = cpool.tile([128, NDB, MEL], F8)
            nc.sync.dma_start(wmel8_t[:], wmel8_d[:])
            bmel_t = cpool.tile([128, NMB], F32)
            nc.sync.dma_start(bmel_t[:], bmel_d[:])

            # ones via memset (no DMA wait) -> PE pre-warm matmuls start
            # immediately, releasing the HAM clock throttle (~3.4us of
            # sustained PE activity) right as the first g chunk lands, and
            # the exp activation-table load (~2.7us) hides under the DMAs.
            ones_t = cpool.tile([1, TC], BF)
            nc.vector.memset(ones_t[:], 1.0)
            warm = cpool.tile([1, 128], F32)
            nc.scalar.activation(warm[:], ones_t[0:1, 0:128],
                                 mybir.ActivationFunctionType.Exp, scale=1.0)
            pwarm = ps.tile([128, TC], F32, tag="warmps", bufs=1)
            for _ in range(8):
                nc.tensor.matmul(pwarm[:], ones_t[0:1, 0:128], ones_t[:],
                                 start=True, stop=True)

            # ---------- software-pipelined schedule ----------
            units = [(b, t) for b in range(B_LOC) for t in range(NTC)]
            st = {}

            def P2(name, shape=None):
                return ps.tile(shape or [128, 2, TC], F32, tag="p2", bufs=3,
                               name=name)

            def emit_const_dma():
                phT8 = cpool.tile([128, NPO, SPK], F8, name="phT8_all")
                nc.scalar.dma_start(phT8[:], phT8_d[:])
                st["phT8"] = phT8
                v8_sb = cpool.tile([128, A, D], F8, name="v8_all")
                nc.scalar.dma_start(v8_sb[:], v8_d[:])
                st["v8"] = v8_sb
                vmel8_sb = cpool.tile([128, BB, MEL], F8, name="vmel8_all")
                nc.scalar.dma_start(vmel8_sb[:], vmel8_d[:])
                st["vmel8"] = vmel8_sb

            def emit_g_dma(u):
                b, t = u
                g8_sb = pool.tile([128, KO, TC], F8, tag="g8", bufs=2,
                                  name=f"g8_{b}_{t}")
                KH2 = KO // 2
                nc.scalar.dma_start(g8_sb[:, 0:KH2, :], g8_d[b, t, :, 0:KH2, :])
                nc.scalar.dma_start(g8_sb[:, KH2:KO, :], g8_d[b, t, :, KH2:KO, :])
                st[("g", u)] = g8_sb

            def emit_kq_half(u, h):
                # all-fp8 DoubleRow: 10 contraction pairs per pb block
                b, t = u
                g8_sb = st[("g", u)]
                if h == 0:
                    st[("kq", u)] = pool.tile([128, NPO, TC], F8, tag="kq",
                                              bufs=2, name=f"kq_{b}_{t}")
                kq_sb = st[("kq", u)]
                with nc.named_scope(f"kq_{b}_{t}_{h}"):
                    pkq = P2(f"pkq_{b}_{t}_{h}")
                    for j in range(2):
                        pb_ = 2 * h + j
                        for k2 in range(KO // 2):
                            nc.tensor.matmul(
                                pkq[:, j, :],
                                wqk8_t[:, 2 * k2:2 * k2 + 2,
                                       pb_ * 128:(pb_ + 1) * 128],
                                g8_sb[:, 2 * k2:2 * k2 + 2, :],
                                start=(k2 == 0), stop=(k2 == KO // 2 - 1),
                                perf_mode=DR,
                            )
                        # drains alternate vector/scalar so both halves of
                        # kq8 land in parallel (att pair 0 waits on them)
                        if pb_ % 2 == 0:
                            nc.vector.tensor_scalar_add(
                                kq_sb[:, pb_, :], pkq[:, j, :],
                                kqb_t[:, pb_:pb_ + 1])
                        else:
                            nc.scalar.activation(
                                kq_sb[:, pb_, :], pkq[:, j, :],
                                mybir.ActivationFunctionType.Identity,
                                bias=kqb_t[:, pb_:pb_ + 1], scale=1.0)

            def emit_att_pair(u, pair):
                b, t = u
                kq_sb = st[("kq", u)]
                phT8 = st["phT8"]
                trip = trips[b]
                if pair == 0:
                    st[("exps", u)] = pool.tile([128, trip, TC], F8,
                                                tag="exps", bufs=2,
                                                padded_shape=[128, max(A, BB), TC],
                                                name=f"exps_{b}_{t}")
                exps = st[("exps", u)]
                with nc.named_scope(f"att_{b}_{t}_{pair}"):
                    pa = P2(f"pa_{b}_{t}_{pair}")
                    for j in range(2):
                        sb = 2 * pair + j
                        so = (SOFF[b] + sb) * 128
                        for p2_ in range(NPO // 2):
                            nc.tensor.matmul(
                                pa[:, j, :],
                                phT8[:, 2 * p2_:2 * p2_ + 2, so:so + 128],
                                kq_sb[:, 2 * p2_:2 * p2_ + 2, :],
                                start=(p2_ == 0), stop=(p2_ == NPO // 2 - 1),
                                perf_mode=DR,
                            )
                        if 2 * pair + 2 > FV[b]:
                            # per-block drain: bias column masks invalid rows
                            # (exp(-100) = 0), patterns differ per core
                            nc.scalar.activation(
                                exps[:, sb:sb + 1, :], pa[:, j:j + 1, :],
                                mybir.ActivationFunctionType.Exp,
                                bias=ebias_t[:, SOFF[b] + sb:SOFF[b] + sb + 1],
                                scale=SCALE / W8S)
                    if 2 * pair + 2 <= FV[b]:
                        # both blocks fully valid on every core: one drain
                        nc.scalar.activation(
                            exps[:, 2 * pair:2 * pair + 2, :], pa[:],
                            mybir.ActivationFunctionType.Exp,
                            bias=ebias_t[:, SOFF[b]:SOFF[b] + 1],
                            scale=SCALE / W8S)

            def emit_den(u, pairs_):
                b, t = u
                trip = trips[b]
                exps = st[("exps", u)]
                if ("pd", u) not in st:
                    st[("pd", u)] = ps.tile([128, TC], F32, tag="den", bufs=1,
                                            name=f"pd_{b}_{t}")
                pd = st[("pd", u)]
                for i in pairs_:
                    nc.tensor.matmul(
                        pd[:],
                        vcol8_t[:, SOFF[b] + 2 * i:SOFF[b] + 2 * i + 2, :],
                        exps[:, 2 * i:2 * i + 2, :],
                        start=(i == 0), stop=(i == trip // 2 - 1),
                        perf_mode=DR,
                    )

            def emit_recip(u):
                # single-pass approx reciprocal (4e-6 rel err, harmless on
                # a softmax denominator); input is the replicated den from
                # either the PE matmul (PSUM) or the DVE tree (SBUF)
                b, t = u
                recipb = pool.tile([128, TC], F32, tag="recipb", bufs=2,
                                   name=f"recipb_{b}_{t}")
                nc.vector.reciprocal_approx_fast(recipb[:], st[("pd", u)][:])
                st[("recipb", u)] = recipb

            def emit_val_mm(u, h):
                # long slot only: value numerator matmuls
                b, t = u
                trip = trips[b]
                exps = st[("exps", u)]
                v8_sb = st["v8"]
                if h == 0:
                    st[("valT", u)] = pool.tile([128, NDB, TC], F8, tag="valT",
                                                bufs=2, name=f"valT_{b}_{t}")
                with nc.named_scope(f"val_{b}_{t}_{h}"):
                    pv2 = P2(f"pv2_{b}_{t}_{h}")
                    st[("pv2", u, h)] = pv2
                    # last exps pair touched as late as possible: its exp
                    # drain races these matmuls at the unit boundary
                    np_ = trip // 2
                    order = [(j, i) for i in range(np_ - 1) for j in range(2)]
                    order += [(0, np_ - 1), (1, np_ - 1)]
                    for j, i in order:
                        db = 2 * h + j
                        nc.tensor.matmul(
                            pv2[:, j, :],
                            v8_sb[:, 2 * i:2 * i + 2,
                                  db * 128:(db + 1) * 128],
                            exps[:, 2 * i:2 * i + 2, :],
                            start=(i == 0), stop=(i == np_ - 1),
                            perf_mode=DR,
                        )

            def emit_val_drain(u, h):
                # valT8 = fp8(VOS*value) = pv2 * recipb
                valT = st[("valT", u)]
                nc.vector.tensor_tensor(
                    valT[:, 2 * h:2 * h + 2, :], st[("pv2", u, h)][:],
                    st[("recipb", u)][:, None, :].to_broadcast((128, 2, TC)),
                    mybir.AluOpType.mult)

            def emit_exn(u):
                # short slot only: exn8 = fp8(EXS*att/den) = exps8 * recipb
                # (two halves so the first fold-out matmul starts earlier)
                b, t = u
                trip = trips[b]
                exps = st[("exps", u)]
                recipb = st[("recipb", u)]
                exn = pool.tile([128, trip, TC], F8, tag="exn", bufs=2,
                                name=f"exn_{b}_{t}")
                h = trip // 2
                for sl in (slice(0, h), slice(h, trip)):
                    nc.vector.tensor_tensor(
                        exn[:, sl, :], exps[:, sl, :],
                        recipb[:, None, :].to_broadcast((128, h, TC)),
                        mybir.AluOpType.mult)
                st[("exn", u)] = exn

            def emit_out_pair(u, pr):
                b, t = u
                with nc.named_scope(f"out_{b}_{t}_{pr}"):
                    po2 = P2(f"po2_{b}_{t}_{pr}")
                    if b == 0:
                        valT = st[("valT", u)]
                        oscale = 1.0 / (VOS * WMS)
                        for j in range(2):
                            mb = 2 * pr + j
                            for i in range(NDB // 2):
                                nc.tensor.matmul(
                                    po2[:, j, :],
                                    wmel8_t[:, 2 * i:2 * i + 2,
                                            mb * 128:(mb + 1) * 128],
                                    valT[:, 2 * i:2 * i + 2, :],
                                    start=(i == 0), stop=(i == NDB // 2 - 1),
                                    perf_mode=DR,
                                )
                    else:
                        exn = st[("exn", u)]
                        vmel8 = st["vmel8"]
                        trip = trips[b]
                        oscale = 1.0 / (VMS * EXS)
                        for j in range(2):
                            mb = 2 * pr + j
                            for i in range(trip // 2):
                                nc.tensor.matmul(
                                    po2[:, j, :],
                                    vmel8[:, 2 * i:2 * i + 2,
                                          mb * 128:(mb + 1) * 128],
                                    exn[:, 2 * i:2 * i + 2, :],
                                    start=(i == 0), stop=(i == trip // 2 - 1),
                                    perf_mode=DR,
                                )
                    out_sb = pool.tile([128, 2, TC], BF, tag="out_sb",
                                       bufs=3, name=f"out_sb_{b}_{t}_{pr}")
                    for j in range(2):
                        mb = 2 * pr + j
                        # short units: all drains on vector so scalar only
                        # runs the exp drains the den matmuls wait on
                        if b == 0 and pr % 2 == 0:
                            nc.scalar.activation(
                                out_sb[:, j, :], po2[:, j, :],
                                mybir.ActivationFunctionType.Identity,
                                bias=bmel_t[:, mb:mb + 1], scale=oscale)
                        else:
                            nc.vector.tensor_scalar(
                                out_sb[:, j, :], po2[:, j, :],
                                oscale, bmel_t[:, mb:mb + 1],
                                mybir.AluOpType.mult, mybir.AluOpType.add)
                    # rows m=f*64+h of this mb pair are contiguous f-major in
                    # the (f h t) output, so one 3-dim store covers both mbs
                    dst = out_d[b, 4 * pr:4 * pr + 4].rearrange(
                        "(j f0) h t -> (f0 h) j t", j=2)
                    nc.sync.dma_start(dst[:, :, t * TC:(t + 1) * TC], out_sb[:])

            # ---------- pipeline driver ----------
            def emit_val_stage(pv):
                # prev's value stage.  For long slots ALL den pairs + recip
                # sit between the two val matmul groups: the den matmuls'
                # waits on the last exp drains are covered by val-mm PE
                # work, and recip (DVE) lands before the val drains enter
                # the DVE queue (in-order engine requirement).
                if pv[0] == 0:
                    emit_val_mm(pv, 0)
                    emit_den(pv, list(range(trips[0] // 2)))
                    emit_recip(pv)
                    emit_val_mm(pv, 1)
                    emit_val_drain(pv, 0)
                    emit_val_drain(pv, 1)
                else:
                    emit_recip(pv)
                    emit_exn(pv)

            emit_g_dma(units[0])
            emit_const_dma()
            prev = None
            for idx, u in enumerate(units):
                trip = trips[u[0]]
                npair = trip // 2
                # long slots: den runs entirely in the next iteration's val
                # stage; shorts finish den here (their den waits are
                # covered by leftover out pairs)
                den_lim = 0 if u[0] == 0 else npair
                if prev is not None:
                    emit_val_stage(prev)
                emit_kq_half(u, 0)
                emit_kq_half(u, 1)
                if idx + 1 < len(units):
                    emit_g_dma(units[idx + 1])
                # interleave att pairs (u) with out pairs (prev)
                last = idx == len(units) - 1
                done_den = 0
                for i in range(max(npair, NMB // 2)):
                    # att first: its exp drains enter the scalar queue
                    # ahead of out-drain work, and out pair i gains PE
                    # cover for its valT/exn drain dependencies
                    if i < npair:
                        emit_att_pair(u, i)
                    if prev is not None and i < NMB // 2:
                        emit_out_pair(prev, i)
                    lag = min(i - 2, den_lim)
                    if 0 < lag > done_den:
                        emit_den(u, list(range(done_den, lag)))
                        done_den = lag
                    if last and u[0] == 1 and i == npair + 1:
                        # final short unit: den/recip/exn early, covered
                        # by prev's leftover out pairs
                        if done_den < den_lim:
                            emit_den(u, list(range(done_den, den_lim)))
                            done_den = den_lim
                        emit_recip(u)
                        emit_exn(u)
                        st[("tail_done", u)] = True
                if done_den < den_lim:
                    emit_den(u, list(range(done_den, den_lim)))
                prev = u
            if ("tail_done", prev) not in st:
                emit_val_stage(prev)
            for pr in range(NMB // 2):
                emit_out_pair(prev, pr)

    nc.compile()
    return nc


def _f8(x):
    return np.ascontiguousarray(
        np.asarray(x, dtype=np.float32).astype(mybir.dt.np(F8)))


def _f32(x):
    return np.ascontiguousarray(np.asarray(x, dtype=np.float32))


def _host_sample(ph_b, g_b, length, Wk, bk, Wv, bv, Wq, bq, Wmel, bmel):
    """Exact f32 reference for one sample -> [64, 20, T]."""
    q = g_b.reshape(CF, T).T @ Wq + bq              # [T, D]
    k = ph_b[:length] @ Wk + bk                     # [L, D]
    att = (q @ k.T) * SCALE                         # [T, L]
    att = att - att.max(axis=1, keepdims=True)
    att = np.exp(att)
    att /= att.sum(axis=1, keepdims=True)
    v = ph_b[:length] @ Wv + bv                     # [L, D]
    out = (att @ v) @ Wmel + bmel                   # [T, MEL]
    return np.ascontiguousarray(
        out.reshape(T, 20, 64).transpose(2, 1, 0))  # [64, 20, T]


def kernel(ph, g, lengths, Wk, bk, Wv, bv, Wq, bq, Wmel, bmel, **_):
    ph = np.asarray(ph, dtype=np.float32)
    g = np.asarray(g, dtype=np.float32)
    lengths = np.asarray(lengths)
    Wk = np.asarray(Wk, dtype=np.float32)
    bk = np.asarray(bk, dtype=np.float32)
    Wv = np.asarray(Wv, dtype=np.float32)
    bv = np.asarray(bv, dtype=np.float32)
    Wq = np.asarray(Wq, dtype=np.float32)
    bq = np.asarray(bq, dtype=np.float32)
    Wmel = np.asarray(Wmel, dtype=np.float32)
    bmel = np.asarray(bmel, dtype=np.float32)

    lens = lengths.astype(np.int64)
    hosted = [b for b in range(B) if int(lens[b]) <= HOST_MAX_LEN]
    nblk = np.maximum(1, -(-lens // 128))          # ceil, >= 1
    order = np.argsort(-lens, kind="stable")
    pairs = [(int(order[i]), int(order[B - 1 - i])) for i in range(N_CORES)]
    A = max(int(nblk[a]) for a, _ in pairs)
    dev_shorts = [b2 for _, b2 in pairs if b2 not in hosted]
    BB = max([int(nblk[b2]) for b2 in dev_shorts], default=2)
    A = min(A + A % 2, S // 128)
    BB = min(BB + BB % 2, S // 128)
    NSBT = A + BB

    # zero ph rows at invalid positions (keeps phT8/v clean; masking
    # itself is done by the exp bias column)
    ph_z = ph.copy()
    for b in range(B):
        ph_z[b, int(lens[b]):, :] = 0.0
    v_full = ph_z.reshape(-1, PH) @ Wv             # [B*S, D], no bv
    v_full = v_full.reshape(B, S, D)

    # host-side prearrangement into device layouts (all fp8)
    g_all = g.reshape(B, KO, 128, NTC, TC).transpose(0, 3, 2, 1, 4)
    g8_h = _f8(g_all)
    phT_h = ph_z.transpose(0, 2, 1).reshape(B, NPO, 128, S).transpose(0, 2, 1, 3)
    wqk = (Wq @ Wk.T) * W8S                        # [CF, PH]
    wqk8_h = _f8(wqk.reshape(KO, 128, PH).transpose(1, 0, 2))
    kqb = (Wk @ bq) * W8S                          # [PH]
    kqb_h = _f32(kqb.reshape(NPO, 128).T)
    wmel8_h = _f8((WMS * Wmel).reshape(NDB, 128, MEL).transpose(1, 0, 2))
    bmel_eff = (bv.astype(np.float64) @ Wmel.astype(np.float64)
                + bmel.astype(np.float64)).astype(np.float32)
    bmel_h = np.ascontiguousarray(bmel_eff.reshape(NMB, 128).T)
    vcol_h = np.zeros((128, NSBT, 128), np.float32)
    vcol_h[:, :A, :] = VS / VOS       # 1.0: valT8 = pv2 / den directly
    vcol_h[:, A:, :] = 1.0 / EXS
    vcol8_h = _f8(vcol_h)

    # leading s-blocks valid on every core (device-resident samples only;
    # hosted lanes' outputs are discarded so their masks don't constrain)
    fva = min(int(lens[a]) for a, _ in pairs) // 128
    fvb = (min([int(lens[b2]) for b2 in dev_shorts], default=0)) // 128
    FV = (min(fva, A), min(fvb, BB))

    nc_key = (A, BB, FV)
    if nc_key not in _NC_CACHE:
        _NC_CACHE[nc_key] = _build_nc(A, BB, FV)
    nc = _NC_CACHE[nc_key]

    in_maps = []
    for c in range(N_CORES):
        sa, sb2 = pairs[c]
        phT_pack = np.zeros((128, NPO, NSBT * 128), np.float32)
        phT_pack[:, :, :A * 128] = phT_h[sa][:, :, :A * 128]
        phT_pack[:, :, A * 128:] = phT_h[sb2][:, :, :BB * 128]
        v8 = _f8(VS * v_full[sa][:A * 128]
                 .reshape(A, 128, D).transpose(1, 0, 2))       # [128, A, D]
        vmel = v_full[sb2][:BB * 128] @ Wmel                   # [BB*128, MEL]
        vmel8 = _f8(VMS * vmel.reshape(BB, 128, MEL).transpose(1, 0, 2))
        ebias = np.zeros((128, NSBT), np.float32)
        pos_a = np.arange(A * 128) < lens[sa]
        ebias[:, :A] = np.where(pos_a.reshape(A, 128).T, 0.0, -100.0)
        pos_b = np.arange(BB * 128) < lens[sb2]
        ebias[:, A:] = np.where(pos_b.reshape(BB, 128).T, 0.0, -100.0)
        in_maps.append({
            "phT8": _f8(phT_pack),
            "g8": np.ascontiguousarray(g8_h[[sa, sb2]]),
            "wqk8": wqk8_h, "v8": v8, "vmel8": vmel8, "wmel8": wmel8_h,
            "bmel": bmel_h, "kqb": kqb_h, "vcol8": vcol8_h, "ebias": ebias,
        })

    res = run_bass_kernel_spmd(nc, in_maps, core_ids=list(range(N_CORES)))
    out = np.empty((B, 64, 20, T), np.float32)
    for c in range(N_CORES):
        sa, sb2 = pairs[c]
        out[sa] = np.asarray(res.results[c]["out"][0],
                             dtype=np.float32).transpose(1, 0, 2)
        out[sb2] = np.asarray(res.results[c]["out"][1],
                              dtype=np.float32).transpose(1, 0, 2)
    for b in hosted:
        out[b] = _host_sample(ph[b], g[b], int(lens[b]),
                              Wk, bk, Wv, bv, Wq, bq, Wmel, bmel)
    return out


# revision 53
# speedup vs baseline: 1.0558x; 1.0558x over previous
"""AVAttention Trainium2 kernel (8 NeuronCores, Bass/Tile).

Reference computation per sample b:
    k   = ph @ Wk + bk                  [S, D]
    q   = g.reshape(CF, T).T @ Wq + bq  [T, D]
    att = softmax(mask(q @ k.T / sqrt(D)))  over S (mask from lengths[b])
    out = (att @ (ph @ Wv + bv)) @ Wmel + bmel -> [64, F, T]

Sharding: data-parallel over batch B=16 across 8 cores, 2 samples per
core (paired long+short by lengths), weights replicated, outputs
concatenated.  No collectives.

Layout: everything on-chip is "transposed" (feature dim on partitions)
so no attention-matrix transposes are ever needed:
    kq[p,t], exps[s,t], valueT[d,t], outT[m,t]

Algebraic folds (host-side, exact):
  * logits = phT^T @ (Wqk^T @ g) with Wqk = Wq @ Wk^T prefolded; bq
    enters as kq += Wk@bq; bk cancels in softmax.
  * v = ph @ Wv on host; bv folded into bmel.
  * short samples: vmel = v @ Wmel prefolded on host, so the value and
    output GEMMs collapse into one: out = (att/den) @ vmel + bmel.
  * samples with len < 128 are computed exactly on the host (f32); the
    device slot still runs but its output is discarded.  This removes
    the error-critical tiny-length samples from the fp8 budget.

Precision: EVERY device GEMM runs fp8e4m3 DoubleRow (2 contraction
rows per PE pass = 2x bf16 throughput): kq (wqk8 x g8), attention
(phT8 x kq8), denominator (vcol8 x exps8), value (v8 x exps8, long
slot), out (wmel8 x valT8 long / vmel8 x exn8 short).  f32 PSUM
everywhere.  Post-softmax fp8 error scales ~1/sqrt(len), so with
len<128 hosted the emulated end-to-end absmax-relative error is
~1.4e-2 vs the 2e-2 gate (host emulator emulate.py, validated to ~4%
against HW on the previous all-bf16+partial-fp8 kernel).

Masking costs nothing: exp() drains with a per-partition bias column
(0 valid / -100 invalid) so invalid positions give exp(-100)=0 exactly;
den then needs no validity column and value/vmel need no zeroed rows.

Scales (all exact powers of two, folded into constants):
  wqk8 = fp8(64*Wqk)           exp scale = SCALE/64
  v8 = fp8(64*v), valT8 = fp8(64*value): vcol8 long entries = 1
  wmel8 = fp8(64*Wmel): long out drain scale = 1/4096
  vmel8 = fp8(16*vmel), exn8 = fp8(64*att/den): vcol8 short = 1/64,
  short out drain scale = 1/1024

Softmax denominator: the den matmul's weights (vcol8) are REPLICATED
across 128 columns so den lands on all PSUM partitions; 1/den is then
a single full-width reciprocal_approx_fast (the plain DVE reciprocal
costs 3.3us regardless of shape; approx is ~5x faster at 18 bits).

Schedule (engines run their queues in order, so emission order = the
dependency schedule): per-unit emission software-pipelines across
units -- prev's value stage (val matmuls sandwiching den+recip so the
den matmuls' exp-drain waits are covered and recip precedes the val
drains in the DVE queue) runs before the current unit's kq; prev's
out pairs interleave with current att pairs; short-unit den lags two
steps behind att under leftover out pairs; the final unit runs
recip/exn inside the interleave so prev's last out pairs cover the
chain.  The val matmuls touch the last exps pair as late as possible
(its exp drain races them at the unit boundary).  kq/out drains
alternate Vector/Scalar.  All matmul PSUMs are [128, 2, TC] tiles (2
banks) rotating 3-deep.  g loads on the Scalar engine's DMA queue.
Output stores in (f h t) bf16, host swaps axes and upcasts.  8
memset-fed warm-up matmuls release the HAM clock throttle during the
initial DMA wait.

Measured on 8 trn2 cores (seed-0 inputs, A=8/BB=4): 186.0us, rel err
1.401e-2 (= the emulate.py prediction; gate 2e-2).  History: 542us ->
354 -> 315 (prior sessions, partial fp8) -> 209 (all-fp8 DR + host
len<128 + fold) -> 189 (replicated den + fast recip + val-first
order) -> 186 (deferred den, val pair ordering, warm-up trim).
PE ~81% busy at the 216ns/matmul stream floor, ~672 matmul slots.
"""

import math

import numpy as np

import concourse.bacc as bacc
import concourse.mybir as mybir
import concourse.tile as tile
from concourse.bass_utils import run_bass_kernel_spmd

B, S, T = 16, 1024, 2048
CF = 2560          # q in_features = C*Fdim = 128*20
KO = CF // 128     # 20 contraction chunks for the kq projection
D = 512            # out_dim (k/q/v width); 4 partition blocks
PH = 512           # ph feature dim; 4 partition blocks
MEL = 1280         # out features; 10 partition blocks
N_CORES = 8
B_LOC = B // N_CORES
SCALE = 1.0 / math.sqrt(D)

TC = 512
NTC = T // TC
NPO = PH // 128
NDB = D // 128
NMB = MEL // 128

F32 = mybir.dt.float32
BF = mybir.dt.bfloat16
F8 = mybir.dt.float8e4
DR = mybir.MatmulPerfMode.DoubleRow

# power-of-two scales keeping every fp8 tensor in e4m3's normal range
W8S = 64.0          # wqk, kq
VS = 64.0           # v8 = fp8(VS*v); == VOS so valT8 = pv2 * (1/den) exactly
VOS = 64.0          # valT8 = fp8(VOS*value)
WMS = 64.0          # wmel8 = fp8(WMS*Wmel); long out scale 1/(VOS*WMS)
VMS = 16.0          # vmel8 = fp8(VMS*vmel)
EXS = 64.0          # exn8 = fp8(EXS*att/den); vcol8 short = 1/EXS
HOST_MAX_LEN = 127  # samples this short are computed on the host

_NC_CACHE = {}


def _build_nc(A, BB, FV=(0, 0)):
    NSBT = A + BB            # total packed s blocks
    SPK = NSBT * 128
    trips = (A, BB)          # att/val block count per sample slot
    SOFF = (0, A)            # block offset of each sample's region
    # FV[b]: number of leading s-blocks fully valid on EVERY core for
    # slot b -> their exp drains can go out as one 2-block activation
    # with a shared (all-zero) bias column.

    nc = bacc.Bacc("TRN2", target_bir_lowering=False,
                   dynamic_dma_scratch_size=256)

    phT8_d = nc.dram_tensor("phT8", [128, NPO, SPK], F8, kind="ExternalInput")
    g8_d = nc.dram_tensor("g8", [B_LOC, NTC, 128, KO, TC], F8, kind="ExternalInput")
    wqk8_d = nc.dram_tensor("wqk8", [128, KO, PH], F8, kind="ExternalInput")
    v8_d = nc.dram_tensor("v8", [128, A, D], F8, kind="ExternalInput")
    vmel8_d = nc.dram_tensor("vmel8", [128, BB, MEL], F8, kind="ExternalInput")
    wmel8_d = nc.dram_tensor("wmel8", [128, NDB, MEL], F8, kind="ExternalInput")
    bmel_d = nc.dram_tensor("bmel", [128, NMB], F32, kind="ExternalInput")
    kqb_d = nc.dram_tensor("kqb", [128, NPO], F32, kind="ExternalInput")
    # [128, NSBT, 128]: den scale replicated across 128 columns, so the
    # den matmul writes its result to ALL output partitions: reciprocal
    # then runs on [128, TC] (full DVE width) and no partition broadcast
    # is needed.  The 128B pair stride also satisfies the s3 ldweights
    # DoubleRow alignment restriction.
    vcol8_d = nc.dram_tensor("vcol8", [128, NSBT, 128], F8, kind="ExternalInput")
    ebias_d = nc.dram_tensor("ebias", [128, NSBT], F32, kind="ExternalInput")
    out_d = nc.dram_tensor("out", [B_LOC, 20, 64, T], BF, kind="ExternalOutput")

    with tile.TileContext(nc) as tc:
        with tc.tile_pool(name="const", bufs=1) as cpool, \
             tc.tile_pool(name="sb", bufs=2) as pool, \
             tc.tile_pool(name="ps", bufs=2, space="PSUM") as ps:

            # ---- constants / weights (resident) ----
            # wqk8 in halves so the first kq group starts after half 0
            # lands; everything else arrives under unit 0.
            KH = KO // 2
            wqk8_t = cpool.tile([128, KO, PH], F8)
            nc.sync.dma_start(wqk8_t[:, 0:KH, :], wqk8_d[:, 0:KH, :])
            kqb_t = cpool.tile([128, NPO], F32)
            nc.sync.dma_start(kqb_t[:], kqb_d[:])
            nc.sync.dma_start(wqk8_t[:, KH:KO, :], wqk8_d[:, KH:KO, :])
            vcol8_t = cpool.tile([128, NSBT, 128], F8)
            nc.sync.dma_start(vcol8_t[:], vcol8_d[:])
            ebias_t = cpool.tile([128, NSBT], F32)
            nc.sync.dma_start(ebias_t[:], ebias_d[:])
            wmel8_t = cpool.tile([128, NDB, MEL], F8)
            nc.sync.dma_start(wmel8_t[:], wmel8_d[:])
            bmel_t = cpool.tile([128, NMB], F32)
            nc.sync.dma_start(bmel_t[:], bmel_d[:])

            # ones via memset (no DMA wait) -> PE pre-warm matmuls start
            # immediately, releasing the HAM clock throttle (~3.4us of
            # sustained PE activity) right as the first g chunk lands, and
            # the exp activation-table load (~2.7us) hides under the DMAs.
            ones_t = cpool.tile([1, TC], BF)
            nc.vector.memset(ones_t[:], 1.0)
            warm = cpool.tile([1, 128], F32)
            nc.scalar.activation(warm[:], ones_t[0:1, 0:128],
                                 mybir.ActivationFunctionType.Exp, scale=1.0)
            pwarm = ps.tile([128, TC], F32, tag="warmps", bufs=1)
            for _ in range(8):
                nc.tensor.matmul(pwarm[:], ones_t[0:1, 0:128], ones_t[:],
                                 start=True, stop=True)

            # ---------- software-pipelined schedule ----------
            units = [(b, t) for b in range(B_LOC) for t in range(NTC)]
            st = {}

            def P2(name, shape=None):
                return ps.tile(shape or [128, 2, TC], F32, tag="p2", bufs=3,
                               name=name)

            def emit_const_dma():
                phT8 = cpool.tile([128, NPO, SPK], F8, name="phT8_all")
                nc.scalar.dma_start(phT8[:], phT8_d[:])
                st["phT8"] = phT8
                v8_sb = cpool.tile([128, A, D], F8, name="v8_all")
                nc.scalar.dma_start(v8_sb[:], v8_d[:])
                st["v8"] = v8_sb
                vmel8_sb = cpool.tile([128, BB, MEL], F8, name="vmel8_all")
                nc.scalar.dma_start(vmel8_sb[:], vmel8_d[:])
                st["vmel8"] = vmel8_sb

            def emit_g_dma(u):
                b, t = u
                g8_sb = pool.tile([128, KO, TC], F8, tag="g8", bufs=2,
                                  name=f"g8_{b}_{t}")
                KH2 = KO // 2
                nc.scalar.dma_start(g8_sb[:, 0:KH2, :], g8_d[b, t, :, 0:KH2, :])
                nc.scalar.dma_start(g8_sb[:, KH2:KO, :], g8_d[b, t, :, KH2:KO, :])
                st[("g", u)] = g8_sb

            def emit_kq_half(u, h):
                # all-fp8 DoubleRow: 10 contraction pairs per pb block
                b, t = u
                g8_sb = st[("g", u)]
                if h == 0:
                    st[("kq", u)] = pool.tile([128, NPO, TC], F8, tag="kq",
                                              bufs=2, name=f"kq_{b}_{t}")
                kq_sb = st[("kq", u)]
                with nc.named_scope(f"kq_{b}_{t}_{h}"):
                    pkq = P2(f"pkq_{b}_{t}_{h}")
                    for j in range(2):
                        pb_ = 2 * h + j
                        for k2 in range(KO // 2):
                            nc.tensor.matmul(
                                pkq[:, j, :],
                                wqk8_t[:, 2 * k2:2 * k2 + 2,
                                       pb_ * 128:(pb_ + 1) * 128],
                                g8_sb[:, 2 * k2:2 * k2 + 2, :],
                                start=(k2 == 0), stop=(k2 == KO // 2 - 1),
                                perf_mode=DR,
                            )
                        # drains alternate vector/scalar so both halves of
                        # kq8 land in parallel (att pair 0 waits on them)
                        if pb_ % 2 == 0:
                            nc.vector.tensor_scalar_add(
                                kq_sb[:, pb_, :], pkq[:, j, :],
                                kqb_t[:, pb_:pb_ + 1])
                        else:
                            nc.scalar.activation(
                                kq_sb[:, pb_, :], pkq[:, j, :],
                                mybir.ActivationFunctionType.Identity,
                                bias=kqb_t[:, pb_:pb_ + 1], scale=1.0)

            def emit_att_pair(u, pair):
                b, t = u
                kq_sb = st[("kq", u)]
                phT8 = st["phT8"]
                trip = trips[b]
                if pair == 0:
                    st[("exps", u)] = pool.tile([128, trip, TC], F8,
                                                tag="exps", bufs=2,
                                                padded_shape=[128, max(A, BB), TC],
                                                name=f"exps_{b}_{t}")
                exps = st[("exps", u)]
                with nc.named_scope(f"att_{b}_{t}_{pair}"):
                    pa = P2(f"pa_{b}_{t}_{pair}")
                    for j in range(2):
                        sb = 2 * pair + j
                        so = (SOFF[b] + sb) * 128
                        for p2_ in range(NPO // 2):
                            nc.tensor.matmul(
                                pa[:, j, :],
                                phT8[:, 2 * p2_:2 * p2_ + 2, so:so + 128],
                                kq_sb[:, 2 * p2_:2 * p2_ + 2, :],
                                start=(p2_ == 0), stop=(p2_ == NPO // 2 - 1),
                                perf_mode=DR,
                            )
                        if 2 * pair + 2 > FV[b]:
                            # per-block drain: bias column masks invalid rows
                            # (exp(-100) = 0), patterns differ per core
                            nc.scalar.activation(
                                exps[:, sb:sb + 1, :], pa[:, j:j + 1, :],
                                mybir.ActivationFunctionType.Exp,
                                bias=ebias_t[:, SOFF[b] + sb:SOFF[b] + sb + 1],
                                scale=SCALE / W8S)
                    if 2 * pair + 2 <= FV[b]:
                        # both blocks fully valid on every core: one drain
                        nc.scalar.activation(
                            exps[:, 2 * pair:2 * pair + 2, :], pa[:],
                            mybir.ActivationFunctionType.Exp,
                            bias=ebias_t[:, SOFF[b]:SOFF[b] + 1],
                            scale=SCALE / W8S)

            def emit_den(u, pairs_):
                b, t = u
                trip = trips[b]
                exps = st[("exps", u)]
                if ("pd", u) not in st:
                    st[("pd", u)] = ps.tile([128, TC], F32, tag="den", bufs=1,
                                            name=f"pd_{b}_{t}")
                pd = st[("pd", u)]
                for i in pairs_:
                    nc.tensor.matmul(
                        pd[:],
                        vcol8_t[:, SOFF[b] + 2 * i:SOFF[b] + 2 * i + 2, :],
                        exps[:, 2 * i:2 * i + 2, :],
                        start=(i == 0), stop=(i == trip // 2 - 1),
                        perf_mode=DR,
                    )

            def emit_recip(u):
                # single-pass approx reciprocal (4e-6 rel err, harmless on
                # a softmax denominator); input is the replicated den from
                # either the PE matmul (PSUM) or the DVE tree (SBUF)
                b, t = u
                recipb = pool.tile([128, TC], F32, tag="recipb", bufs=2,
                                   name=f"recipb_{b}_{t}")
                nc.vector.reciprocal_approx_fast(recipb[:], st[("pd", u)][:])
                st[("recipb", u)] = recipb

            def emit_val_mm(u, h):
                # long slot only: value numerator matmuls
                b, t = u
                trip = trips[b]
                exps = st[("exps", u)]
                v8_sb = st["v8"]
                if h == 0:
                    st[("valT", u)] = pool.tile([128, NDB, TC], F8, tag="valT",
                                                bufs=2, name=f"valT_{b}_{t}")
                with nc.named_scope(f"val_{b}_{t}_{h}"):
                    pv2 = P2(f"pv2_{b}_{t}_{h}")
                    st[("pv2", u, h)] = pv2
                    # last exps pair touched as late as possible: its exp
                    # drain races these matmuls at the unit boundary
                    np_ = trip // 2
                    order = [(j, i) for i in range(np_ - 1) for j in range(2)]
                    order += [(0, np_ - 1), (1, np_ - 1)]
                    for j, i in order:
                        db = 2 * h + j
                        nc.tensor.matmul(
                            pv2[:, j, :],
                            v8_sb[:, 2 * i:2 * i + 2,
                                  db * 128:(db + 1) * 128],
                            exps[:, 2 * i:2 * i + 2, :],
                            start=(i == 0), stop=(i == np_ - 1),
                            perf_mode=DR,
                        )

            def emit_val_drain(u, h):
                # valT8 = fp8(VOS*value) = pv2 * recipb
                valT = st[("valT", u)]
                nc.vector.tensor_tensor(
                    valT[:, 2 * h:2 * h + 2, :], st[("pv2", u, h)][:],
                    st[("recipb", u)][:, None, :].to_broadcast((128, 2, TC)),
                    mybir.AluOpType.mult)

            def emit_exn(u):
                # short slot only: exn8 = fp8(EXS*att/den) = exps8 * recipb
                # (two halves so the first fold-out matmul starts earlier)
                b, t = u
                trip = trips[b]
                exps = st[("exps", u)]
                recipb = st[("recipb", u)]
                exn = pool.tile([128, trip, TC], F8, tag="exn", bufs=2,
                                name=f"exn_{b}_{t}")
                h = trip // 2
                for sl in (slice(0, h), slice(h, trip)):
                    nc.vector.tensor_tensor(
                        exn[:, sl, :], exps[:, sl, :],
                        recipb[:, None, :].to_broadcast((128, h, TC)),
                        mybir.AluOpType.mult)
                st[("exn", u)] = exn

            def emit_out_pair(u, pr):
                b, t = u
                with nc.named_scope(f"out_{b}_{t}_{pr}"):
                    po2 = P2(f"po2_{b}_{t}_{pr}")
                    if b == 0:
                        valT = st[("valT", u)]
                        oscale = 1.0 / (VOS * WMS)
                        for j in range(2):
                            mb = 2 * pr + j
                            for i in range(NDB // 2):
                                nc.tensor.matmul(
                                    po2[:, j, :],
                                    wmel8_t[:, 2 * i:2 * i + 2,
                                            mb * 128:(mb + 1) * 128],
                                    valT[:, 2 * i:2 * i + 2, :],
                                    start=(i == 0), stop=(i == NDB // 2 - 1),
                                    perf_mode=DR,
                                )
                    else:
                        exn = st[("exn", u)]
                        vmel8 = st["vmel8"]
                        trip = trips[b]
                        oscale = 1.0 / (VMS * EXS)
                        for j in range(2):
                            mb = 2 * pr + j
                            for i in range(trip // 2):
                                nc.tensor.matmul(
                                    po2[:, j, :],
                                    vmel8[:, 2 * i:2 * i + 2,
                                          mb * 128:(mb + 1) * 128],
                                    exn[:, 2 * i:2 * i + 2, :],
                                    start=(i == 0), stop=(i == trip // 2 - 1),
                                    perf_mode=DR,
                                )
                    out_sb = pool.tile([128, 2, TC], BF, tag="out_sb",
                                       bufs=3, name=f"out_sb_{b}_{t}_{pr}")
                    for j in range(2):
                        mb = 2 * pr + j
                        if pr % 2 == 0:
                            nc.scalar.activation(
                                out_sb[:, j, :], po2[:, j, :],
                                mybir.ActivationFunctionType.Identity,
                                bias=bmel_t[:, mb:mb + 1], scale=oscale)
                        else:
                            nc.vector.tensor_scalar(
                                out_sb[:, j, :], po2[:, j, :],
                                oscale, bmel_t[:, mb:mb + 1],
                                mybir.AluOpType.mult, mybir.AluOpType.add)
                    # rows m=f*64+h of this mb pair are contiguous f-major in
                    # the (f h t) output, so one 3-dim store covers both mbs
                    dst = out_d[b, 4 * pr:4 * pr + 4].rearrange(
                        "(j f0) h t -> (f0 h) j t", j=2)
                    nc.sync.dma_start(dst[:, :, t * TC:(t + 1) * TC], out_sb[:])

            # ---------- pipeline driver ----------
            def emit_val_stage(pv):
                # prev's value stage.  For long slots ALL den pairs + recip
                # sit between the two val matmul groups: the den matmuls'
                # waits on the last exp drains are covered by val-mm PE
                # work, and recip (DVE) lands before the val drains enter
                # the DVE queue (in-order engine requirement).
                if pv[0] == 0:
                    emit_val_mm(pv, 0)
                    emit_den(pv, list(range(trips[0] // 2)))
                    emit_recip(pv)
                    emit_val_mm(pv, 1)
                    emit_val_drain(pv, 0)
                    emit_val_drain(pv, 1)
                else:
                    emit_recip(pv)
                    emit_exn(pv)

            emit_g_dma(units[0])
            emit_const_dma()
            prev = None
            for idx, u in enumerate(units):
                trip = trips[u[0]]
                npair = trip // 2
                # long slots: den runs entirely in the next iteration's val
                # stage; shorts finish den here (their den waits are
                # covered by leftover out pairs)
                den_lim = 0 if u[0] == 0 else npair
                if prev is not None:
                    emit_val_stage(prev)
                emit_kq_half(u, 0)
                emit_kq_half(u, 1)
                if idx + 1 < len(units):
                    emit_g_dma(units[idx + 1])
                # interleave att pairs (u) with out pairs (prev)
                last = idx == len(units) - 1
                done_den = 0
                for i in range(max(npair, NMB // 2)):
                    if prev is not None and i < NMB // 2:
                        emit_out_pair(prev, i)
                    if i < npair:
                        emit_att_pair(u, i)
                    lag = min(i - 2, den_lim)
                    if 0 < lag > done_den:
                        emit_den(u, list(range(done_den, lag)))
                        done_den = lag
                    if last and u[0] == 1 and i == npair + 1:
                        # final short unit: den/recip/exn early, covered
                        # by prev's leftover out pairs
                        if done_den < den_lim:
                            emit_den(u, list(range(done_den, den_lim)))
                            done_den = den_lim
                        emit_recip(u)
                        emit_exn(u)
                        st[("tail_done", u)] = True
                if done_den < den_lim:
                    emit_den(u, list(range(done_den, den_lim)))
                prev = u
            if ("tail_done", prev) not in st:
                emit_val_stage(prev)
            for pr in range(NMB // 2):
                emit_out_pair(prev, pr)

    nc.compile()
    return nc


def _f8(x):
    return np.ascontiguousarray(
        np.asarray(x, dtype=np.float32).astype(mybir.dt.np(F8)))


def _f32(x):
    return np.ascontiguousarray(np.asarray(x, dtype=np.float32))


def _host_sample(ph_b, g_b, length, Wk, bk, Wv, bv, Wq, bq, Wmel, bmel):
    """Exact f32 reference for one sample -> [64, 20, T]."""
    q = g_b.reshape(CF, T).T @ Wq + bq              # [T, D]
    k = ph_b[:length] @ Wk + bk                     # [L, D]
    att = (q @ k.T) * SCALE                         # [T, L]
    att = att - att.max(axis=1, keepdims=True)
    att = np.exp(att)
    att /= att.sum(axis=1, keepdims=True)
    v = ph_b[:length] @ Wv + bv                     # [L, D]
    out = (att @ v) @ Wmel + bmel                   # [T, MEL]
    return np.ascontiguousarray(
        out.reshape(T, 20, 64).transpose(2, 1, 0))  # [64, 20, T]


def kernel(ph, g, lengths, Wk, bk, Wv, bv, Wq, bq, Wmel, bmel, **_):
    ph = np.asarray(ph, dtype=np.float32)
    g = np.asarray(g, dtype=np.float32)
    lengths = np.asarray(lengths)
    Wk = np.asarray(Wk, dtype=np.float32)
    bk = np.asarray(bk, dtype=np.float32)
    Wv = np.asarray(Wv, dtype=np.float32)
    bv = np.asarray(bv, dtype=np.float32)
    Wq = np.asarray(Wq, dtype=np.float32)
    bq = np.asarray(bq, dtype=np.float32)
    Wmel = np.asarray(Wmel, dtype=np.float32)
    bmel = np.asarray(bmel, dtype=np.float32)

    lens = lengths.astype(np.int64)
    hosted = [b for b in range(B) if int(lens[b]) <= HOST_MAX_LEN]
    nblk = np.maximum(1, -(-lens // 128))          # ceil, >= 1
    order = np.argsort(-lens, kind="stable")
    pairs = [(int(order[i]), int(order[B - 1 - i])) for i in range(N_CORES)]
    A = max(int(nblk[a]) for a, _ in pairs)
    dev_shorts = [b2 for _, b2 in pairs if b2 not in hosted]
    BB = max([int(nblk[b2]) for b2 in dev_shorts], default=2)
    A = min(A + A % 2, S // 128)
    BB = min(BB + BB % 2, S // 128)
    NSBT = A + BB

    # zero ph rows at invalid positions (keeps phT8/v clean; masking
    # itself is done by the exp bias column)
    ph_z = ph.copy()
    for b in range(B):
        ph_z[b, int(lens[b]):, :] = 0.0
    v_full = ph_z.reshape(-1, PH) @ Wv             # [B*S, D], no bv
    v_full = v_full.reshape(B, S, D)

    # host-side prearrangement into device layouts (all fp8)
    g_all = g.reshape(B, KO, 128, NTC, TC).transpose(0, 3, 2, 1, 4)
    g8_h = _f8(g_all)
    phT_h = ph_z.transpose(0, 2, 1).reshape(B, NPO, 128, S).transpose(0, 2, 1, 3)
    wqk = (Wq @ Wk.T) * W8S                        # [CF, PH]
    wqk8_h = _f8(wqk.reshape(KO, 128, PH).transpose(1, 0, 2))
    kqb = (Wk @ bq) * W8S                          # [PH]
    kqb_h = _f32(kqb.reshape(NPO, 128).T)
    wmel8_h = _f8((WMS * Wmel).reshape(NDB, 128, MEL).transpose(1, 0, 2))
    bmel_eff = (bv.astype(np.float64) @ Wmel.astype(np.float64)
                + bmel.astype(np.float64)).astype(np.float32)
    bmel_h = np.ascontiguousarray(bmel_eff.reshape(NMB, 128).T)
    vcol_h = np.zeros((128, NSBT, 128), np.float32)
    vcol_h[:, :A, :] = VS / VOS       # 1.0: valT8 = pv2 / den directly
    vcol_h[:, A:, :] = 1.0 / EXS
    vcol8_h = _f8(vcol_h)

    # leading s-blocks valid on every core (device-resident samples only;
    # hosted lanes' outputs are discarded so their masks don't constrain)
    fva = min(int(lens[a]) for a, _ in pairs) // 128
    fvb = (min([int(lens[b2]) for b2 in dev_shorts], default=0)) // 128
    FV = (min(fva, A), min(fvb, BB))

    nc_key = (A, BB, FV)
    if nc_key not in _NC_CACHE:
        _NC_CACHE[nc_key] = _build_nc(A, BB, FV)
    nc = _NC_CACHE[nc_key]

    in_maps = []
    for c in range(N_CORES):
        sa, sb2 = pairs[c]
        phT_pack = np.zeros((128, NPO, NSBT * 128), np.float32)
        phT_pack[:, :, :A * 128] = phT_h[sa][:, :, :A * 128]
        phT_pack[:, :, A * 128:] = phT_h[sb2][:, :, :BB * 128]
        v8 = _f8(VS * v_full[sa][:A * 128]
                 .reshape(A, 128, D).transpose(1, 0, 2))       # [128, A, D]
        vmel = v_full[sb2][:BB * 128] @ Wmel                   # [BB*128, MEL]
        vmel8 = _f8(VMS * vmel.reshape(BB, 128, MEL).transpose(1, 0, 2))
        ebias = np.zeros((128, NSBT), np.float32)
        pos_a = np.arange(A * 128) < lens[sa]
        ebias[:, :A] = np.where(pos_a.reshape(A, 128).T, 0.0, -100.0)
        pos_b = np.arange(BB * 128) < lens[sb2]
        ebias[:, A:] = np.where(pos_b.reshape(BB, 128).T, 0.0, -100.0)
        in_maps.append({
            "phT8": _f8(phT_pack),
            "g8": np.ascontiguousarray(g8_h[[sa, sb2]]),
            "wqk8": wqk8_h, "v8": v8, "vmel8": vmel8, "wmel8": wmel8_h,
            "bmel": bmel_h, "kqb": kqb_h, "vcol8": vcol8_h, "ebias": ebias,
        })

    res = run_bass_kernel_spmd(nc, in_maps, core_ids=list(range(N_CORES)))
    out = np.empty((B, 64, 20, T), np.float32)
    for c in range(N_CORES):
        sa, sb2 = pairs[c]
        out[sa] = np.asarray(res.results[c]["out"][0],
                             dtype=np.float32).transpose(1, 0, 2)
        out[sb2] = np.asarray(res.results[c]["out"][1],
                              dtype=np.float32).transpose(1, 0, 2)
    for b in hosted:
        out[b] = _host_sample(ph[b], g[b], int(lens[b]),
                              Wk, bk, Wv, bv, Wq, bq, Wmel, bmel)
    return out
